# revision 3
# baseline (speedup 1.0000x reference)
"""Trainium2 Bass kernel for a seq2seq CandlestickLSTM.

Model (per reference): 2-layer LSTM encoder over S=64 steps, then a
2-layer LSTM decoder run autoregressively for T=32 steps with an MLP
head (Linear(H,H/2) -> ReLU -> Linear(H/2,OUT) -> Sigmoid) whose output
feeds back as the next decoder input.

Sharding: pure data parallel over 8 NeuronCores -- batch 4096 -> 512
rows per core; all weights replicated. No collectives needed.

On-core layout: feature-major ("transposed"): activations h, c live as
[128 partitions, HT, Bc] where hidden tile k occupies page k. Matmuls
compute z.T = W @ input.T via out = lhsT.T @ rhs with host-pre-packed
weights.

Precision: the large K=256 hidden-state projections run in fp8e4m3 with
MatmulPerfMode.DoubleRow (2 fp8 K-rows per PE cycle, K=256 in a single
matmul). Weights and h are pre-scaled by 32 (products by 1024) to keep
fp8 values in the normal range; the 1/1024 unwind rides the activation
instruction's free scale operand. The tiny K=4 input projections and
the MLP second layer stay bf16 (weights pre-scaled by 1024 where they
accumulate into scaled PSUM). PSUM accumulation and pre-activation z
are fp32; gates and c are bf16. Validated end-to-end rel err ~1e-3 vs
the fp32 reference (gate 2e-2).
"""

import numpy as np
import ml_dtypes
from contextlib import ExitStack

import concourse.bass as bass
import concourse.tile as tile
from concourse import bacc, mybir
from concourse.bass_utils import run_bass_kernel_spmd

NCORES = 8
B, S, IN, H, OUT = 4096, 64, 4, 256, 4
BC = B // NCORES          # 512 batch rows per core
HT = H // 128             # 2 hidden 128-tiles
GT = 4 * H // 128         # 8 gate M-tiles
HH = H // 2               # 128 (MLP hidden)
F32 = mybir.dt.float32
BF16 = mybir.dt.bfloat16
F8 = mybir.dt.float8e4
AF = mybir.ActivationFunctionType
ALU = mybir.AluOpType
DR = mybir.MatmulPerfMode.DoubleRow

_BF = ml_dtypes.bfloat16
_F8 = ml_dtypes.float8_e4m3

SW = 32.0                 # fp8 weight pre-scale
SH = 32.0                 # fp8 hidden-state pre-scale
ZS = SW * SH              # PSUM z scale (unwound in the activation)

_cache = {}


def _pack_whT_f8(W):
    """W [4H, K] (K%256==0) -> DoubleRow pack [128, K/128, 4H] fp8 (x SW).

    [p, kt, m] = W.T[p + 128*kt, m]; the DR lhsT for gate m-tile m is
    arr[:, :, 128m : 128m+128].
    """
    M, K = W.shape
    assert K % 256 == 0
    kt = K // 128
    WT = np.ascontiguousarray(W.T) * SW          # [K, M]
    arr = WT.reshape(kt, 128, M).transpose(1, 0, 2)
    return np.ascontiguousarray(arr).astype(_F8)


def _pack_bias(b):
    """b [4H] -> [128, GT] with column m = b[128m:128(m+1)]."""
    return np.ascontiguousarray(b.reshape(GT, 128).T).astype(np.float32)


def _build(T, lstm_bias_flags, repeats=1):
    """Build + compile the per-core program. lstm_bias_flags: 4 bools for
    (enc0, enc1, dec0, dec1) biases being nonzero."""
    nc = bacc.Bacc(
        "TRN2",
        target_bir_lowering=False,
        debug=False,
        enable_asserts=False,
    )

    def din(name, shape, dt):
        return nc.dram_tensor(name, shape, dt, kind="ExternalInput").ap()

    d_xT = din("xT", [IN, S * BC], BF16)
    d_we0x = din("we0x", [IN, 4 * H], BF16)          # x ZS
    d_we0h = din("we0h", [128, HT, 4 * H], F8)       # x SW
    d_we1x = din("we1x", [128, HT, 4 * H], F8)
    d_we1h = din("we1h", [128, HT, 4 * H], F8)
    d_wd0x = din("wd0x", [IN, 4 * H], BF16)          # x ZS
    d_wd0h = din("wd0h", [128, HT, 4 * H], F8)
    d_wd1x = din("wd1x", [128, HT, 4 * H], F8)
    d_wd1h = din("wd1h", [128, HT, 4 * H], F8)
    d_wp1 = din("wp1", [128, HT, HH], F8)
    d_wp2 = din("wp2", [HH, OUT], BF16)
    d_bp1 = din("bp1", [HH, 1], F32)
    d_bp2 = din("bp2", [OUT, 1], F32)
    d_lb = [None] * 4
    for li, flag in enumerate(lstm_bias_flags):
        if flag:
            d_lb[li] = din(f"lstmbias{li}", [128, GT], F32)

    out = nc.dram_tensor("out", [BC, T, OUT], F32, kind="ExternalOutput").ap()
    out_r = out.rearrange("b t c -> c t b")  # [OUT, T, BC] view for DMA scatter

    INV = 1.0 / ZS

    with tile.TileContext(nc) as tc, ExitStack() as ctx:
        persist = ctx.enter_context(tc.tile_pool(name="persist", bufs=1))

        def load(name, dram_ap, shape, dt):
            t = persist.tile(shape, dt, name=name)
            nc.sync.dma_start(t[:], dram_ap[:])
            return t

        s_we0x = load("s_we0x", d_we0x, [IN, 4 * H], BF16)
        s_we0h = load("s_we0h", d_we0h, [128, HT, 4 * H], F8)
        s_we1x = load("s_we1x", d_we1x, [128, HT, 4 * H], F8)
        s_we1h = load("s_we1h", d_we1h, [128, HT, 4 * H], F8)
        s_wd0x = load("s_wd0x", d_wd0x, [IN, 4 * H], BF16)
        s_wd0h = load("s_wd0h", d_wd0h, [128, HT, 4 * H], F8)
        s_wd1x = load("s_wd1x", d_wd1x, [128, HT, 4 * H], F8)
        s_wd1h = load("s_wd1h", d_wd1h, [128, HT, 4 * H], F8)
        s_wp1 = load("s_wp1", d_wp1, [128, HT, HH], F8)
        s_wp2 = load("s_wp2", d_wp2, [HH, OUT], BF16)
        s_bp1 = load("s_bp1", d_bp1, [HH, 1], F32)
        s_bp2 = load("s_bp2", d_bp2, [OUT, 1], F32)
        s_lb = [None] * 4
        for li in range(4):
            if d_lb[li] is not None:
                s_lb[li] = load(f"s_lstmbias{li}", d_lb[li], [128, GT], F32)

        # x.T staged in chunks so step 0 doesn't wait on the whole tensor.
        s_xT = persist.tile([IN, S * BC], BF16, name="s_xT")
        XCH = 8
        chw = S * BC // XCH
        for ci in range(XCH):
            nc.sync.dma_start(
                s_xT[:, ci * chw : (ci + 1) * chw],
                d_xT[:, ci * chw : (ci + 1) * chw],
            )

        zp = ctx.enter_context(tc.tile_pool(name="zp", bufs=4, space="PSUM"))
        gp = ctx.enter_context(tc.tile_pool(name="gp", bufs=2))
        sp = ctx.enter_context(tc.tile_pool(name="sp", bufs=2))

        # gate order in z rows: i, f, g, o (PyTorch) -> pair index p below.
        # Emission order f, i, g, o lets DVE start t1 = sig_f * c_prev while
        # the g/o matmuls still stream.
        GATES = (("f", 1, AF.Sigmoid), ("i", 0, AF.Sigmoid),
                 ("g", 2, AF.Tanh), ("o", 3, AF.Sigmoid))

        def cell(tag, layer, h_chunks, x_chunk, c_prev, bias_t, first):
            """Emit one LSTM cell.

            h_chunks: list of (w3d, h3d) DoubleRow K=256 contributions
              (weight tile [128, HT, 4H] fp8, state tile [128, HT, BC] fp8).
            x_chunk: None or (wx, rhs_ap, kpart) bf16 contribution with
              K=kpart (x or pred input; weights pre-scaled by ZS).
            Returns (h_new[fp8 x SH, 3D], c_new[bf16]).
            """
            n_mm = len(h_chunks) + (1 if x_chunk is not None else 0)
            gate_sb = {}
            for gname, p, func in GATES:
                z = zp.tile([128, HT, BC], F32, tag="z", name=f"z_{tag}_{gname}")
                for j in range(HT):
                    m = 2 * p + j
                    dst = z[:, j, :]
                    mi = 0
                    if x_chunk is not None:
                        wx, rhs_ap, kpart = x_chunk
                        nc.tensor.matmul(
                            dst, wx[0:kpart, 128 * m : 128 * m + 128], rhs_ap,
                            start=True, stop=(n_mm == 1),
                        )
                        mi = 1
                    for (w3, h3) in h_chunks:
                        nc.tensor.matmul(
                            dst,
                            w3[:, :, 128 * m : 128 * m + 128],
                            h3[:, :, :],
                            start=(mi == 0), stop=(mi == n_mm - 1),
                            perf_mode=DR,
                        )
                        mi += 1
                g = gp.tile([128, HT, BC], BF16, tag=f"gate_{gname}",
                            name=f"gt_{tag}_{gname}")
                if bias_t is None:
                    nc.scalar.activation(g[:], z[:], func, scale=INV)
                else:
                    for j in range(HT):
                        m = 2 * p + j
                        nc.scalar.activation(
                            g[:, j, :], z[:, j, :], func,
                            bias=bias_t[:, m : m + 1], scale=INV,
                        )
                gate_sb[gname] = g

            c_new = sp.tile([128, HT, BC], BF16, tag=f"c{layer}", name=f"c_{tag}")
            h_new = sp.tile([128, HT, BC], F8, tag=f"h{layer}", name=f"h_{tag}")
            tc_t = gp.tile([128, HT, BC], BF16, tag="tanh_c", name=f"tc_{tag}")
            if first:
                nc.vector.tensor_mul(c_new[:], gate_sb["i"][:], gate_sb["g"][:])
            else:
                t1 = gp.tile([128, HT, BC], BF16, tag="t1", name=f"t1_{tag}")
                t2 = gp.tile([128, HT, BC], BF16, tag="t2", name=f"t2_{tag}")
                nc.vector.tensor_mul(t1[:], gate_sb["f"][:], c_prev[:])
                nc.vector.tensor_mul(t2[:], gate_sb["i"][:], gate_sb["g"][:])
                nc.vector.tensor_add(c_new[:], t1[:], t2[:])
            nc.scalar.activation(tc_t[:], c_new[:], AF.Tanh)
            # h (x SH) in fp8 for the next DoubleRow matmul
            nc.vector.scalar_tensor_tensor(
                h_new[:], gate_sb["o"][:], SH, tc_t[:], ALU.mult, ALU.mult,
            )
            return h_new, c_new

        def emit_forward():
            h0 = c0 = h1 = c1 = None
            # ---------------- encoder ----------------
            for t in range(S):
                first = t == 0
                xt = s_xT[:, BC * t : BC * (t + 1)]
                h0, c0 = cell(
                    f"e0_{t}", 0,
                    [] if first else [(s_we0h, h0)],
                    (s_we0x, xt, IN), c0, s_lb[0], first,
                )
                # h1-part first (older dependency), h0-part last
                h_chunks = ([(s_we1x, h0)] if first
                            else [(s_we1h, h1), (s_we1x, h0)])
                h1, c1 = cell(f"e1_{t}", 1, h_chunks, None, c1, s_lb[1], first)

            # ---------------- decoder ----------------
            pred_bf = None
            for t in range(T):
                if t == 0:
                    xt = s_xT[:, BC * (S - 1) : BC * S]
                else:
                    xt = pred_bf[:]
                h0, c0 = cell(
                    f"d0_{t}", 0, [(s_wd0h, h0)], (s_wd0x, xt, IN),
                    c0, s_lb[2], False,
                )
                h1, c1 = cell(
                    f"d1_{t}", 1, [(s_wd1h, h1), (s_wd1x, h0)], None,
                    c1, s_lb[3], False,
                )

                # MLP head: relu(Wp1 @ h1 + bp1) -> sigmoid(Wp2 @ . + bp2)
                m1_ps = zp.tile([HH, BC], F32, tag="z", name=f"m1ps_{t}")
                nc.tensor.matmul(
                    m1_ps[:], s_wp1[:, :, :], h1[:, :, :],
                    start=True, stop=True, perf_mode=DR,
                )
                m1_sb = gp.tile([HH, BC], BF16, tag="m1sb", name=f"m1sb_{t}")
                nc.scalar.activation(m1_sb[:], m1_ps[:], AF.Relu,
                                     bias=s_bp1[:, 0:1], scale=INV)
                m2_ps = zp.tile([OUT, BC], F32, tag="z", name=f"m2ps_{t}")
                nc.tensor.matmul(m2_ps[:], s_wp2[:], m1_sb[:], start=True, stop=True)
                pred_f = gp.tile([OUT, BC], F32, tag="predf", name=f"predf_{t}")
                nc.scalar.activation(pred_f[:], m2_ps[:], AF.Sigmoid,
                                     bias=s_bp2[:, 0:1])
                nc.sync.dma_start(out_r[:, t, :], pred_f[:])
                if t < T - 1:
                    pred_bf = gp.tile([OUT, BC], BF16, tag="predbf",
                                      name=f"predbf_{t}")
                    nc.vector.tensor_copy(pred_bf[:], pred_f[:])

        for _rep in range(repeats):
            emit_forward()

    nc.compile()
    return nc


def _prep_shared(inputs):
    f32 = lambda k: np.asarray(inputs[k], np.float32)
    shared = {
        "we0x": (np.ascontiguousarray(f32("enc_Wih0").T) * ZS).astype(_BF),
        "we0h": _pack_whT_f8(f32("enc_Whh0")),
        "we1x": _pack_whT_f8(f32("enc_Wih1")),
        "we1h": _pack_whT_f8(f32("enc_Whh1")),
        "wd0x": (np.ascontiguousarray(f32("dec_Wih0").T) * ZS).astype(_BF),
        "wd0h": _pack_whT_f8(f32("dec_Whh0")),
        "wd1x": _pack_whT_f8(f32("dec_Wih1")),
        "wd1h": _pack_whT_f8(f32("dec_Whh1")),
        "wp1": _pack_whT_f8(f32("Wp1")),
        "wp2": np.ascontiguousarray(f32("Wp2").T).astype(_BF),
        "bp1": np.ascontiguousarray(f32("bp1").reshape(HH, 1)),
        "bp2": np.ascontiguousarray(f32("bp2").reshape(OUT, 1)),
    }
    lstm_biases = [f32("enc_b0"), f32("enc_b1"), f32("dec_b0"), f32("dec_b1")]
    flags = tuple(bool(np.any(b != 0)) for b in lstm_biases)
    for li, (b, flag) in enumerate(zip(lstm_biases, flags)):
        if flag:
            shared[f"lstmbias{li}"] = _pack_bias(b)
    return shared, flags


def _make_in_maps(inputs):
    x = np.asarray(inputs["x"], np.float32)
    assert x.shape == (B, S, IN), x.shape
    shared, _ = _prep_shared(inputs)
    in_maps = []
    for c in range(NCORES):
        xc = x[c * BC : (c + 1) * BC]                       # [BC, S, IN]
        xT = np.ascontiguousarray(xc.transpose(2, 1, 0))    # [IN, S, BC]
        in_maps.append({"xT": xT.reshape(IN, S * BC).astype(_BF), **shared})
    return in_maps


def kernel(**inputs):
    T = int(np.asarray(inputs["target_length"]))

    _, flags = _prep_shared(inputs)
    key = (T, flags)
    if key not in _cache:
        _cache[key] = _build(T, flags)
    nc = _cache[key]

    in_maps = _make_in_maps(inputs)

    res = run_bass_kernel_spmd(nc, in_maps, list(range(NCORES)))
    return np.concatenate(
        [res.results[i]["out"] for i in range(NCORES)], axis=0
    ).astype(np.float32)
